# revision 13
# baseline (speedup 1.0000x reference)
"""Trainium2 Bass kernel for nn_DaleDendriticMLP (topk_masking).

Tensor-parallel over the 2048 hidden units across 8 NeuronCores (256
units per core). K-winners and abs-argmax boundary gaps on this problem
are ~4e-7 relative, so all value-bearing matmuls must be fp32-exact-ish:
they are computed as 3-pass fp16 limb products (a = ah + al with fp16
limbs capturing 22 bits; d = ah@bh + ah@bl + al@bh, products exact in
fp32 PSUM, residual ~2^-23) which measures ~2.5e-7 end-to-end like the
HW fp32 mode but streams at 1 cycle/col instead of fp32's 4 (2 HW
passes x 2 cycles/col), and FWL (fp16 weight loads) hides LDWEIGHTS.

Weights (W*maskW, segW*maskS) are premasked and limb-split on the host
(weight preprocessing; no data-dependent host compute). x and context
are limb-split on device.

Per layer each core extracts its local top-32 gated values + unit
indices per sample and AllGathers them per batch-half (24KB payloads,
each launched as soon as that half's top-k finishes). Every core finds
the exact per-row 102nd-largest of the merged 256 candidates with 13
rounds of max8 + mask-subtract removal (no match-register reloads),
scatters the fp32 candidate values as u16 pairs into a dense row
(gpsimd), thresholds, and PE-transposes to [unit, batch].

Schedule highlights: the two batch halves run as staggered pipelines
from ff onward (ff-b0 -> gate/topk-b0 -> AG-b0 while ff-b1 streams);
the L2 dendrite matmuls fill the PE during L1's AllGather + merge +
scatter window; seg-max reduces are placed so they never delay either
the post-AG chain or the next dendrite half (max on DVE, min as
tensor_tensor trees on GPSIMD); the Dale head streams wex with wix
fused as column 100 and applies the rank-1 Wei@Wix correction as one
DVE op.
"""

import os

os.environ.pop("JAX_PLATFORMS", None)
if not os.environ.get("BASS_TRACE"):
    os.environ["BASS_NEVER_TRACE"] = "1"

import numpy as np

import concourse.bacc as bacc
import concourse.tile as tile
import concourse.mybir as mybir
from concourse.bass_utils import run_bass_kernel_spmd

R = 8                    # cores
B = 256                  # batch
HID = 2048
U = HID // R             # 256 units per core
D_IN = 2048
D_CTX = 1024
KI = D_IN // 128         # 16 input K-chunks
KC = D_CTX // 128        # 8 context K-chunks
KH = HID // 128          # 16 hidden K-chunks
NSEG = 10
OUT = 100
KWIN = 102
LOC_ROUNDS = 4           # local top-32 per core
MERGE_ROUNDS = 13        # top-104 of merged 256
NEG = -1.0e30
BIG = 1.0e30
VAL_N = 8 * LOC_ROUNDS   # 32 candidates per row per core per bt
# per-bt payload: 128 rows x 32 vals (f32) + 128 x 32 idx (u16 in f32 slots)
PAYB = 128 * VAL_N + 128 * VAL_N // 2    # 4096 + 2048 = 6144 f32

f32 = mybir.dt.float32
f16 = mybir.dt.float16
u16 = mybir.dt.uint16
i16 = mybir.dt.int16
X = mybir.AxisListType.X
ALU = mybir.AluOpType
AF = mybir.ActivationFunctionType

_CACHE = {}
LAST_RESULT = None
ABL = set(x for x in os.environ.get("ABL", "").split(",") if x)


def _build():
    nc = bacc.Bacc(
        "TRN2",
        target_bir_lowering=False,
        debug=False,
        enable_asserts=False,
        num_devices=R,
    )

    dram = {}

    def din(name, shape, dt=f32):
        dram[name] = nc.dram_tensor(name, shape, dt, kind="ExternalInput")
        return dram[name]

    din("xT", [D_IN, B])
    din("cT", [D_CTX, B])
    for L in (1, 2):
        din(f"sgh{L}", [D_CTX, 2, NSEG, 128], f16)
        din(f"sgl{L}", [D_CTX, 2, NSEG, 128], f16)
        din(f"wh{L}", [D_IN if L == 1 else HID, U], f16)
        din(f"wl{L}", [D_IN if L == 1 else HID, U], f16)
        din(f"b{L}", [1, U])
    din("wexT", [HID, OUT])
    din("wixT", [HID, 1])
    din("weiT", [1, OUT])
    din("bout", [1, OUT])
    out_d = nc.dram_tensor("out", [B, OUT], f32, kind="ExternalOutput")

    ident_d = nc.inline_tensor(np.eye(128, dtype=np.float32), "ident")
    ones_d = nc.inline_tensor(np.ones((1, 128), np.float32), "ones_row")
    # u16-scatter offsets: candidate block r (32 cands) lands in quarter r//2
    # at u16 base 512*(r%2); even/odd slots hold lo/hi halves
    j = np.arange(R * VAL_N)
    roffE_np = np.broadcast_to(
        (512 * ((j // VAL_N) % 2)).astype(np.uint16), (128, R * VAL_N))
    roffE_d = nc.inline_tensor(np.ascontiguousarray(roffE_np), "roffE")
    roffO_d = nc.inline_tensor(np.ascontiguousarray(roffE_np + 1), "roffO")

    gath_g = {
        (L, bt): nc.dram_tensor(f"gath_g{L}_{bt}", [R * PAYB], f32,
                                kind="Internal", addr_space="Shared")
        for L in (1, 2) for bt in range(2)
    }
    groups = [list(range(R))]

    with tile.TileContext(nc) as tc:
        with (
            tc.tile_pool(name="pa", bufs=1) as pa,          # persistent SBUF
            tc.tile_pool(name="pin", bufs=2) as pin,        # xT / nxt (f32)
            tc.tile_pool(name="plb", bufs=2) as plb,        # ff lhs limbs f16
            tc.tile_pool(name="pw", bufs=1) as pw,          # ff W limbs f16
            tc.tile_pool(name="psg", bufs=2) as psg,        # seg limb slices
            tc.tile_pool(name="pdram", bufs=2, space="DRAM") as pdram,
            tc.tile_pool(name="pp_d", bufs=1, space="PSUM") as pp_d,
            tc.tile_pool(name="pp_y", bufs=1, space="PSUM") as pp_y,
            tc.tile_pool(name="pp_m", bufs=1, space="PSUM") as pp_m,
        ):
            from concourse import library_config

            nc.gpsimd.load_library(library_config.local_scatter)

            # context streams on the (otherwise empty) gpsimd queue so its
            # limbs are ready before the first seg slice lands
            cT = pa.tile([128, KC, B], f32, tag="cT")
            nc.gpsimd.dma_start(
                cT[:], dram["cT"][:].rearrange("(k p) b -> p k b", p=128))

            ident = pa.tile([128, 128], f32, tag="ident")
            nc.sync.dma_start(ident[:], ident_d[:])
            ones = pa.tile([1, 128], f32, tag="ones")
            nc.sync.dma_start(ones[:], ones_d[:])

            # PE warmup: dummy matmuls so HAM reaches K=8/8 before the
            # first dendrite matmul issues
            wps = pp_m.tile([128, 4, 128], f32, tag="psm", name="wps")
            for _ in range(24):
                nc.tensor.matmul(wps[:, 0, :], lhsT=ident[:], rhs=ident[:],
                                 start=True, stop=True)

            # context limbs (device): ch + cl = cT to 22 bits
            ch = pa.tile([128, KC, B], f16, tag="ch")
            nc.vector.tensor_copy(ch[:], cT[:])
            chf = pa.tile([128, KC, B], f32, tag="hback", name="chf")
            nc.scalar.copy(chf[:], ch[:])
            cl = pa.tile([128, KC, B], f16, tag="cl")
            nc.vector.tensor_tensor(cl[:], cT[:], chf[:], op=ALU.subtract)

            # x limbs (device); xT streams on the ACT queue (needed at ff1)
            xT = pin.tile([128, KI, B], f32, tag="xin")
            nc.scalar.dma_start(
                xT[:], dram["xT"][:].rearrange("(k p) b -> p k b", p=128))
            xh = plb.tile([128, KI, B], f16, tag="fh")
            nc.vector.tensor_copy(xh[:], xT[:])
            xhf = pa.tile([128, KI, B], f32, tag="hback")
            nc.scalar.copy(xhf[:], xh[:])
            xl = plb.tile([128, KI, B], f16, tag="fl")
            nc.vector.tensor_tensor(xl[:], xT[:], xhf[:], op=ALU.subtract)

            bigt = pa.tile([128, R * VAL_N], f32, tag="bigt")
            nc.vector.memset(bigt[:], BIG)

            # ---------- building blocks ----------
            def emit_seg_dma(L, uh, g2, eng):
                rows = slice(256 * g2, 256 * (g2 + 1))
                sgh_t = psg.tile([128, 2, NSEG * 128], f16, tag="sgh")
                eng.dma_start(
                    sgh_t[:].rearrange("p k (s u) -> p k s u", s=NSEG),
                    dram[f"sgh{L}"][rows, uh].rearrange(
                        "(k p) s u -> p k s u", p=128))
                sgl_t = psg.tile([128, 2, NSEG * 128], f16, tag="sgl")
                eng.dma_start(
                    sgl_t[:].rearrange("p k (s u) -> p k s u", s=NSEG),
                    dram[f"sgl{L}"][rows, uh].rearrange(
                        "(k p) s u -> p k s u", p=128))
                return sgh_t, sgl_t

            def emit_dend_mm(L, uh):
                """Dendrite matmuls for one 128-unit half; returns psums."""
                eng0 = nc.gpsimd if (L == 1 and uh == 0) else nc.sync
                sgs = [emit_seg_dma(L, uh, g2, eng0 if g2 == 0 else nc.sync)
                       for g2 in range(4)]
                dps = [pp_d.tile([128, NSEG, 128], f32, tag=f"d{bt}",
                                 name=f"d{L}{uh}{bt}") for bt in range(2)]
                dfl = [dps[bt][:].rearrange("p s u -> p (s u)")
                       for bt in range(2)]
                for g2 in range(4):
                    sgh_t, sgl_t = sgs[g2]
                    for k2 in range(2):
                        k = 2 * g2 + k2
                        for bt in range(2):
                            bsl = slice(128 * bt, 128 * (bt + 1))
                            for c0, ncols in ((0, 512), (512, 512),
                                              (1024, 256)):
                                dst = dfl[bt][:, c0:c0 + ncols]
                                nc.tensor.matmul(
                                    dst, lhsT=ch[:, k, bsl],
                                    rhs=sgh_t[:, k2, c0:c0 + ncols],
                                    start=(k == 0), stop=False)
                                nc.tensor.matmul(
                                    dst, lhsT=ch[:, k, bsl],
                                    rhs=sgl_t[:, k2, c0:c0 + ncols],
                                    start=False, stop=False)
                                nc.tensor.matmul(
                                    dst, lhsT=cl[:, k, bsl],
                                    rhs=sgh_t[:, k2, c0:c0 + ncols],
                                    start=False,
                                    stop=(k == KC - 1))
                return dps

            def emit_red_max(dps, uh, maxd):
                """Seg-max on DVE (strided reduce straight from PSUM)."""
                for bt in range(2):
                    v = dps[bt][:].rearrange("p s u -> p u s")
                    col = U * bt + 128 * uh
                    nc.vector.tensor_reduce(
                        maxd[:, col:col + 128], v, axis=X, op=ALU.max)

            def emit_red_min(dps, uh, mind):
                """Seg-min on DVE (strided reduce straight from PSUM)."""
                for bt in range(2):
                    v = dps[bt][:].rearrange("p s u -> p u s")
                    col = U * bt + 128 * uh
                    nc.vector.tensor_reduce(
                        mind[:, col:col + 128], v, axis=X, op=ALU.min)

            def emit_ff_w(L, nk):
                wh_t = pw.tile([128, nk, U], f16, tag="wh", name=f"wh{L}")
                nc.sync.dma_start(
                    wh_t[:],
                    dram[f"wh{L}"][:].rearrange("(k p) u -> p k u", p=128))
                wl_t = pw.tile([128, nk, U], f16, tag="wl", name=f"wl{L}")
                nc.sync.dma_start(
                    wl_t[:],
                    dram[f"wl{L}"][:].rearrange("(k p) u -> p k u", p=128))
                b_sb = pa.tile([1, U], f32, tag="bias", name=f"bias{L}")
                nc.scalar.dma_start(b_sb[:], dram[f"b{L}"][:])
                return wh_t, wl_t, b_sb

            def emit_ff_bt(L, bt, fh, fl, nk, wh_t, wl_t, b_sb, y_all):
                bsl = slice(128 * bt, 128 * (bt + 1))
                yp = pp_y.tile([128, 2, U], f32, tag="yp", name=f"yp{L}{bt}")
                for k in range(nk):
                    nc.tensor.matmul(yp[:, 0, :], lhsT=fh[:, k, bsl],
                                     rhs=wh_t[:, k, :],
                                     start=(k == 0), stop=False)
                    nc.tensor.matmul(yp[:, 0, :], lhsT=fh[:, k, bsl],
                                     rhs=wl_t[:, k, :],
                                     start=False, stop=False)
                    nc.tensor.matmul(yp[:, 0, :], lhsT=fl[:, k, bsl],
                                     rhs=wh_t[:, k, :],
                                     start=False, stop=False)
                nc.tensor.matmul(yp[:, 0, :], lhsT=ones[:], rhs=b_sb[:],
                                 start=False, stop=True)
                nc.scalar.copy(y_all[:, U * bt:U * (bt + 1)], yp[:, 0, :])

            def emit_gate_bt(L, bt, maxd, mind, y_all, yg, g, ga, gb, sig):
                sl = slice(U * bt, U * (bt + 1))
                nc.vector.tensor_tensor(g[:, sl], maxd[:, sl], mind[:, sl],
                                        op=ALU.add)
                nc.vector.scalar_tensor_tensor(
                    ga[:, sl], g[:, sl], 0.0, maxd[:, sl],
                    op0=ALU.is_ge, op1=ALU.mult)
                nc.vector.scalar_tensor_tensor(
                    gb[:, sl], g[:, sl], 0.0, mind[:, sl],
                    op0=ALU.is_lt, op1=ALU.mult)
                nc.vector.tensor_tensor(ga[:, sl], ga[:, sl], gb[:, sl],
                                        op=ALU.add)
                nc.scalar.activation(sig[:, sl], ga[:, sl], AF.Sigmoid)
                nc.vector.tensor_tensor(yg[:, sl], y_all[:, sl], sig[:, sl],
                                        op=ALU.mult)

            def emit_topk_ag(L, bt, yg):
                """Local top-32 of this bt half, payload DMA + AllGather."""
                vals_c = pa.tile([128, VAL_N], f32, tag=f"vals{bt}",
                                 name=f"vals{L}{bt}")
                idx_c = pa.tile([128, VAL_N], u16, tag=f"idxc{bt}",
                                name=f"idxc{L}{bt}")
                sc = yg[:, U * bt:U * (bt + 1)]
                for r in range(LOC_ROUNDS):
                    v8 = vals_c[:, 8 * r:8 * (r + 1)]
                    nc.vector.max(v8, sc)
                    nc.vector.max_index(idx_c[:, 8 * r:8 * (r + 1)], v8, sc)
                    if r < LOC_ROUNDS - 1:
                        nc.vector.match_replace(sc, v8, sc, NEG)
                pay = pdram.tile([PAYB], f32, tag="pay")
                nc.gpsimd.dma_start(
                    pay[0:128 * VAL_N].rearrange("(p j) -> p j", p=128),
                    vals_c[:])
                nc.gpsimd.dma_start(
                    pay[128 * VAL_N:].bitcast(u16).rearrange(
                        "(p j) -> p j", p=128),
                    idx_c[:])
                if "nocc" in ABL:
                    nc.gpsimd.dma_start(gath_g[(L, bt)][0:PAYB], pay[:])
                else:
                    nc.gpsimd.collective_compute(
                        "AllGather", ALU.bypass, replica_groups=groups,
                        ins=[pay.opt()], outs=[gath_g[(L, bt)][:]])

            def emit_gather_scatter(L, bt, hrec):
                """Post-AG: two gathers (merge copy + scatter copy), index
                expansion and raw-value scatter, all on GPSIMD."""
                gath = gath_g[(L, bt)]
                src_v = gath[:].rearrange("(r q) -> r q", q=PAYB)\
                    [:, 0:128 * VAL_N].rearrange("r (p j) -> p r j", p=128)
                merged = pa.tile([128, R * VAL_N], f32, tag=f"mrg{bt}",
                                 name=f"mrg{L}{bt}")
                nc.gpsimd.dma_start(
                    merged[:].rearrange("p (r j) -> p r j", r=R), src_v)
                cands = pa.tile([128, R * VAL_N], f32, tag=f"cnd{bt}",
                                name=f"cnd{L}{bt}")
                nc.gpsimd.dma_start(
                    cands[:].rearrange("p (r j) -> p r j", r=R), src_v)
                idxg = pa.tile([128, R * VAL_N], u16, tag=f"idg{bt}",
                               name=f"idg{L}{bt}")
                nc.gpsimd.dma_start(
                    idxg[:].rearrange("p (r j) -> p r j", r=R),
                    gath[:].rearrange("(r q) -> r q", q=PAYB)
                    [:, 128 * VAL_N:].bitcast(u16).rearrange(
                        "r (p j) -> p r j", p=128))
                idxe = pa.tile([128, 2 * R * VAL_N], i16, tag=f"ide{bt}",
                               name=f"ide{L}{bt}")
                iev = idxe[:].rearrange("p (j t) -> p t j", t=2)
                nc.vector.scalar_tensor_tensor(
                    iev[:, 0, :], idxg[:], 2.0, roffE[:],
                    op0=ALU.mult, op1=ALU.add)
                nc.vector.scalar_tensor_tensor(
                    iev[:, 1, :], idxg[:], 2.0, roffO[:],
                    op0=ALU.mult, op1=ALU.add)
                c16 = cands[:].bitcast(u16)
                for q in range(4):
                    nc.gpsimd.local_scatter(
                        hrec[:, bt, 512 * q:512 * (q + 1)].bitcast(u16),
                        c16[:, 128 * q:128 * (q + 1)],
                        idxe[:, 128 * q:128 * (q + 1)],
                        channels=128, num_elems=1024, num_idxs=128)
                return merged

            def emit_merge(L, bt, merged):
                """Rank-102 threshold: max8 rounds with mask-subtract
                removal (no match-register loads). Returns thr AP."""
                mv = pa.tile([128, 8 * MERGE_ROUNDS], f32, tag=f"mv{bt}",
                             name=f"mv{L}{bt}")
                mtmp = pa.tile([128, R * VAL_N], f32, tag=f"mtmp{bt}",
                               name=f"mtmp{L}{bt}")
                for r in range(MERGE_ROUNDS):
                    v8 = mv[:, 8 * r:8 * (r + 1)]
                    nc.vector.max(v8, merged[:])
                    if r < MERGE_ROUNDS - 1:
                        nc.vector.scalar_tensor_tensor(
                            mtmp[:], merged[:], v8[:, 7:8], bigt[:],
                            op0=ALU.is_ge, op1=ALU.mult)
                        nc.vector.tensor_tensor(
                            merged[:], merged[:], mtmp[:], op=ALU.subtract)
                return mv[:, KWIN - 1:KWIN]

            def emit_thresh(L, bt, thr, hrec):
                for q in range(4):
                    hq = hrec[:, bt, 512 * q:512 * (q + 1)]
                    nc.vector.scalar_tensor_tensor(
                        hq, hq, thr, hq, op0=ALU.is_ge, op1=ALU.mult)

            def emit_trans_bt(L, hrec, nxt, bt):
                """Transpose all 16 unit-chunks of one bt half into
                nxt[:, :, bt]; 4 psum groups alternating banks."""
                for c4 in range(4):
                    if c4 % 2 == 0:
                        buf = pp_m.tile([128, 4, 128], f32, tag="psm",
                                        name=f"tp{L}{bt}{c4}")
                    else:
                        b2 = pp_y.tile([128, 2, U], f32, tag="yp",
                                       name=f"tp{L}{bt}{c4}")
                        buf = b2[:].rearrange("p a (c d) -> p (a c) d",
                                              d=128)
                    for ci in range(4):
                        c = 4 * c4 + ci
                        nc.tensor.transpose(
                            buf[:, ci, :],
                            hrec[:, bt, 128 * c:128 * (c + 1)], ident[:])
                    nc.scalar.copy(
                        nxt[:, 4 * c4:4 * c4 + 4, 128 * bt:128 * (bt + 1)],
                        buf[:])

            def emit_limb_bt(nxt, hh, hl, bt):
                sl = slice(128 * bt, 128 * (bt + 1))
                nc.vector.tensor_copy(hh[:, :, sl], nxt[:, :, sl])
                hbf = pa.tile([128, KH, 128], f32, tag="hback",
                              name=f"hback{bt}")
                nc.scalar.copy(hbf[:], hh[:, :, sl])
                nc.vector.tensor_tensor(hl[:, :, sl], nxt[:, :, sl], hbf[:],
                                        op=ALU.subtract)

            # ================= schedule =================
            # --- L1 dendrites (reduces inline: DVE/GPSIMD idle here) ---
            maxd1 = pa.tile([128, 2 * U], f32, tag="maxd")
            mind1 = pa.tile([128, 2 * U], f32, tag="mind")
            for uh in range(2):
                dps = emit_dend_mm(1, uh)
                emit_red_max(dps, uh, maxd1)
                emit_red_min(dps, uh, mind1)

            # scatter offsets (needed only at the post-AG chain)
            roffE = pa.tile([128, R * VAL_N], u16, tag="roffE")
            nc.sync.dma_start(roffE[:], roffE_d[:])
            roffO = pa.tile([128, R * VAL_N], u16, tag="roffO")
            nc.sync.dma_start(roffO[:], roffO_d[:])

            # --- L1 ff/gate/topk per bt half, AG fired per half ---
            wh1, wl1, b1 = emit_ff_w(1, KI)
            y1 = pa.tile([128, 2 * U], f32, tag="y_all")
            yg1 = pa.tile([128, 2 * U], f32, tag="yg")
            g1t = pa.tile([128, 2 * U], f32, tag="g")
            ga1 = pa.tile([128, 2 * U], f32, tag="ga")
            gb1 = pa.tile([128, 2 * U], f32, tag="gb")
            sig1 = pa.tile([128, 2 * U], f32, tag="sig")
            for bt in range(2):
                emit_ff_bt(1, bt, xh, xl, KI, wh1, wl1, b1, y1)
                emit_gate_bt(1, bt, maxd1, mind1, y1, yg1,
                             g1t, ga1, gb1, sig1)
                emit_topk_ag(1, bt, yg1)

            # --- L2 dendrites fill the PE during AG1 + merge + scatter ---
            maxd2 = pa.tile([128, 2 * U], f32, tag="maxd2")
            mind2 = pa.tile([128, 2 * U], f32, tag="mind2")
            dps20 = emit_dend_mm(2, 0)
            dps21 = emit_dend_mm(2, 1)

            hrec1 = pa.tile([128, 2, HID], f32, tag="hrec", name="hrec1")
            mrg1b0 = emit_gather_scatter(1, 0, hrec1)     # gpsimd
            thr1b0 = emit_merge(1, 0, mrg1b0)             # DVE
            # uh0 reduces: max on DVE (before merge-b1), min on GPSIMD
            # (before scatter-b1) so dend2-uh1's psum WAR clears promptly
            emit_red_max(dps20, 0, maxd2)
            emit_red_min(dps20, 0, mind2)
            mrg1b1 = emit_gather_scatter(1, 1, hrec1)
            emit_thresh(1, 0, thr1b0, hrec1)
            thr1b1 = emit_merge(1, 1, mrg1b1)
            emit_thresh(1, 1, thr1b1, hrec1)
            emit_red_max(dps21, 1, maxd2)
            emit_red_min(dps21, 1, mind2)

            # head weights (tail-only; loaded mid-kernel)
            wexf = pa.tile([128, KH, OUT + 1], f32, tag="wexf")
            nc.sync.dma_start(
                wexf[:, :, 0:OUT],
                dram["wexT"][:].rearrange("(k p) o -> p k o", p=128))
            nc.sync.dma_start(
                wexf[:, :, OUT:OUT + 1],
                dram["wixT"][:].rearrange("(k p) o -> p k o", p=128))
            wei = pa.tile([1, OUT], f32, tag="wei")
            nc.sync.dma_start(wei[:], dram["weiT"][:])
            boutx = pa.tile([1, OUT + 1], f32, tag="boutx")
            nc.vector.memset(boutx[:], 0.0)
            nc.sync.dma_start(boutx[:, 0:OUT], dram["bout"][:])

            # --- L1->L2 transition + L2 ff/gate/topk, per-bt pipeline ---
            wh2, wl2, b2 = emit_ff_w(2, KH)
            nxt = pin.tile([128, KH, B], f32, tag="xin", name="h1T")
            hh = plb.tile([128, KH, B], f16, tag="fh", name="h1h")
            hl = plb.tile([128, KH, B], f16, tag="fl", name="h1l")
            y2 = pa.tile([128, 2 * U], f32, tag="y_all", name="y2")
            yg2 = pa.tile([128, 2 * U], f32, tag="yg", name="yg2")
            for bt in range(2):
                emit_trans_bt(1, hrec1, nxt, bt)
                emit_limb_bt(nxt, hh, hl, bt)
                emit_ff_bt(2, bt, hh, hl, KH, wh2, wl2, b2, y2)
                emit_gate_bt(2, bt, maxd2, mind2, y2, yg2,
                             g1t, ga1, gb1, sig1)
                emit_topk_ag(2, bt, yg2)

            # wei broadcast to [128, OUT] via PE for the DVE rank-1 fixup
            wps2 = pp_m.tile([128, 4, 128], f32, tag="psm", name="wps2")
            nc.tensor.matmul(wps2[:, 0, 0:OUT], lhsT=ones[:], rhs=wei[:],
                             start=True, stop=True)
            wei128 = pa.tile([128, OUT], f32, tag="wei128")
            nc.scalar.copy(wei128[:], wps2[:, 0, 0:OUT])

            # --- L2 tail: per-bt merge -> thresh -> transpose -> head ---
            hrec2 = pa.tile([128, 2, HID], f32, tag="hrec", name="hrec2")
            h2T = pin.tile([128, KH, B], f32, tag="xin", name="h2T")
            hpd = pp_d.tile([128, NSEG, 128], f32, tag="d0", name="head_ps")
            hps = [hpd[:, 0, 0:OUT + 1], hpd[:, 4, 0:OUT + 1]]
            for bt in range(2):
                mrg2 = emit_gather_scatter(2, bt, hrec2)
                thr2 = emit_merge(2, bt, mrg2)
                emit_thresh(2, bt, thr2, hrec2)
                emit_trans_bt(2, hrec2, h2T, bt)
                bsl = slice(128 * bt, 128 * (bt + 1))
                for k in range(KH):
                    nc.tensor.matmul(
                        hps[bt], lhsT=h2T[:, k, bsl],
                        rhs=wexf[:, k, :], start=(k == 0), stop=False)
                nc.tensor.matmul(hps[bt], lhsT=ones[:], rhs=boutx[:],
                                 start=False, stop=True)
                ob = pa.tile([128, OUT], f32, tag=f"ob{bt}")
                nc.scalar.copy(ob[:], hps[bt][:, 0:OUT])
                nhx = pa.tile([128, 1], f32, tag=f"nhx{bt}")
                nc.scalar.mul(nhx[:], hps[bt][:, OUT:OUT + 1], -1.0)
                nc.vector.scalar_tensor_tensor(
                    ob[:], wei128[:], nhx[:], ob[:],
                    op0=ALU.mult, op1=ALU.add)
                nc.sync.dma_start(out_d[128 * bt:128 * (bt + 1)], ob[:])

    nc.compile()
    return nc


def _prep_inputs(inputs):
    """Host prep: shard, transpose, premask weights, fp16 limb split."""
    np32 = lambda a: np.ascontiguousarray(np.asarray(a, dtype=np.float32))

    def limbs(a):
        h = a.astype(np.float16)
        l = (a - h.astype(np.float32)).astype(np.float16)
        return np.ascontiguousarray(h), np.ascontiguousarray(l)

    common = {
        "xT": np.ascontiguousarray(np32(inputs["x"]).T),
        "cT": np.ascontiguousarray(np32(inputs["context"]).T),
        "wexT": np.ascontiguousarray(np32(inputs["Wex_out"]).T),
        "wixT": np.ascontiguousarray(np32(inputs["Wix_out"]).T),
        "weiT": np.ascontiguousarray(np32(inputs["Wei_out"]).T),
        "bout": np32(inputs["b_out"]).reshape(1, OUT),
    }
    in_maps = []
    for r in range(R):
        sh = slice(r * U, (r + 1) * U)
        m = dict(common)
        for L, (Wn, bn, sgn, mwn, msn) in {
            1: ("W1", "b1", "segW1", "maskW1", "maskS1"),
            2: ("W2", "b2", "segW2", "maskW2", "maskS2"),
        }.items():
            Wm = np32(inputs[Wn])[sh] * np32(inputs[mwn])[sh]   # [256, nin]
            sgm = np32(inputs[sgn])[sh] * np32(inputs[msn])[sh]

            def seg_layout(a):
                # [u=256, s=10, c=1024] -> [c, uh=2, s, u128]
                t = a.transpose(2, 1, 0)                    # [c, s, u]
                t = t.reshape(D_CTX, NSEG, 2, 128)
                return np.ascontiguousarray(t.transpose(0, 2, 1, 3))

            sgh, sgl = limbs(seg_layout(sgm))
            wh, wl = limbs(np.ascontiguousarray(Wm.T))
            m[f"sgh{L}"] = sgh
            m[f"sgl{L}"] = sgl
            m[f"wh{L}"] = wh
            m[f"wl{L}"] = wl
            m[f"b{L}"] = np32(inputs[bn])[sh].reshape(1, U)
        in_maps.append(m)
    return in_maps


def kernel(**inputs) -> np.ndarray:
    global LAST_RESULT
    if "nc" not in _CACHE:
        _CACHE["nc"] = _build()
    in_maps = _prep_inputs(inputs)
    res = run_bass_kernel_spmd(_CACHE["nc"], in_maps, core_ids=list(range(R)))
    LAST_RESULT = res
    return np.asarray(res.results[0]["out"], dtype=np.float32)
